# revision 39
# baseline (speedup 1.0000x reference)
"""Trainium2 Bass kernel for nn_Attention_76089640616322.

Bahdanau-style attention:
  B, S, HE, DOUT = 32, 4096, 512, 512  (HD = 1024)
  energy = tanh(concat([context, broadcast(output)], -1) @ W1.T)   [B,S,HE]
  attn   = softmax(energy @ W2.T, axis=S)                           [B,1,S]
  mix    = attn @ context                                           [B,1,HE]
  out    = tanh(concat([mix, output], -1) @ Wout.T + bout)          [B,1,HE]

Sharding: pure data parallel, batch dim across 8 cores (4 batches/core),
weights replicated.

Structure (per core, context batch kept SBUF-resident in fp8):
- All weight-only layout transforms (W1/Wout transposes, fp8 scaling and
  DoubleRow pairing of W1's context half, W2/bout columnization) are done
  host-side in make_in_maps; the device loads them in their final layout.
- The broadcast `output` columns of the concat make W1 @ concat(...) =
  W1[:, :HE] @ context + (W1[:, HE:] @ output_b); the second term is a
  per-batch constant fused as a per-partition bias into the tanh (ACT).
- context is cast fp32->fp8e4 during the HBM load (SWDGE cast DMA) with a
  per-partition-contiguous rearrange (s = k*512 + 4p + ss) so each DMA
  descriptor covers an 8 KiB HBM-side run. The fp8 copy serves both the
  energy matmul and the mix contraction (mix contributes only ~2% of the
  final pre-tanh signal, so fp8 there is harmless; measured 4.3e-4 rel
  err end-to-end in numpy).
- ctx is PE-transposed in fp8 (1 cycle/row). The PE writes fp8 transpose
  results 2 bytes apart at 4-byte aligned starts (HW rules), so the
  psum/sbuf tiles carry a trailing stride-2 dim; the PSUM->SBUF copyback
  moves value+pad byte pairs as uint16 (keeps the DVE 2-byte 2X mode) and
  the DoubleRow rhs reads the fp8 values through the stride-2 view.
- energyT = W1cT.T @ ctxT runs as fp8 DoubleRow matmuls (2 k-tiles per
  instruction, 2 MACs/cell/cycle) with fp32 PSUM accumulation. W1cT is
  pre-scaled by 64 into fp8 (keeps the sigma=0.02 weights out of the fp8
  denormal range); the tanh ACT applies the compensating 1/64 scale.
- energy psum + tanh are fused over PAIRS of s-blocks (2-bank psum tile,
  one ACT op per pair) to amortize the ACT per-op PSUM-access overhead.
- The W2 logit dot and the softmax-weighted mix contraction both use the
  "stationary flip": the large tile (tanhT / ctx chunk) rides the
  weight-load stream and the 1-wide vector is the moving operand, so their
  outputs land directly in column form and cost no 512-cycle PE streams.
- Softmax is computed unnormalized (|logit| <= ||W2||_1 ~ 8, exp safe in
  fp32; exp on ACT with a free per-partition accum for the denominator);
  the 1/Z and the Wout output-half contribution (+bout) are both folded
  into the final tanh via its scale/bias operands, so the reciprocal chain
  runs in parallel with the mix->Wout matmuls.
- Batch tails are deferred one batch so their serial chain overlaps the
  next batch's block pipeline.
"""

from contextlib import ExitStack

import numpy as np

import concourse.bass as bass
import concourse.tile as tile
from concourse import bacc, mybir
from concourse._compat import with_exitstack
from concourse.masks import make_identity

B, S, HE, DOUT = 32, 4096, 512, 512
HD = HE + DOUT
NCORES = 8
BC = B // NCORES  # batches per core

F32 = mybir.dt.float32
BF16 = mybir.dt.bfloat16
FP8 = mybir.dt.float8e4
AF = mybir.ActivationFunctionType
DOUBLE_ROW = mybir.MatmulPerfMode.DoubleRow
W1SCALE = 64.0

NSBLK = 8       # s-blocks per batch (512 s each)
SBLK = S // NSBLK   # 512
NSS = SBLK // 128   # 4 subtiles of 128 s per block
NEC = HE // 128     # 4 e-chunks
NDC = HE // 128     # 4 d-chunks for the context half of W1
SCHUNKS = S // 128  # 32 s-chunks of 128 per batch


@with_exitstack
def attention_kernel(ctx: ExitStack, tc: tile.TileContext, out_ap, ins):
    nc = tc.nc

    ctx_ap = ins["context"]    # [BC, S, HE] f32
    w1c8_ap = ins["w1c8"]      # [2, 128, 2, HE] fp8: scaled W1cT, DR-paired
    w1oT_ap = ins["w1oT"]      # [128, NDC, HE] f32: W1 output-half, transposed
    woutT_ap = ins["woutT"]    # [128, 8, HE] f32: Wout transposed
    w2col_ap = ins["w2col"]    # [128, NEC] bf16: W2 columnized
    boutcol_ap = ins["boutcol"]  # [128, NEC] f32: bout columnized
    outpcol_ap = ins["outpcol"]  # [128, BC*4] f32: output rows columnized

    const = ctx.enter_context(tc.tile_pool(name="const", bufs=1))
    ctx_pool = ctx.enter_context(tc.tile_pool(name="ctx", bufs=18))
    ctxT_pool = ctx.enter_context(tc.tile_pool(name="ctxT", bufs=8))
    tanh_pool = ctx.enter_context(tc.tile_pool(name="tanh", bufs=8))
    small = ctx.enter_context(tc.tile_pool(name="small", bufs=2))

    psum_tp = ctx.enter_context(tc.tile_pool(name="ptp", bufs=2, space="PSUM"))
    psum_en = ctx.enter_context(tc.tile_pool(name="pen", bufs=2, space="PSUM"))
    psum_pcol = ctx.enter_context(tc.tile_pool(name="ppcol", bufs=1, space="PSUM"))
    psum_misc = ctx.enter_context(tc.tile_pool(name="pmisc", bufs=1, space="PSUM"))

    # ---- weight/constant loads (all in final layout, HWDGE), ordered by
    # first use: identities and the energy weights gate batch 0's pipeline,
    # tail-only tensors (ones, woutT, boutcol) ride at the back ----
    id128q = const.tile([128, 128], FP8)
    nc.sync.dma_start(out=id128q, in_=ins["id128q"])
    w1c8sb = []
    for pair in range(2):
        t = const.tile([128, 2, HE], FP8, tag=f"w1c8_{pair}")
        nc.sync.dma_start(out=t, in_=w1c8_ap[pair])
        w1c8sb.append(t)
    outpcol = const.tile([128, BC * 4], F32)
    nc.sync.dma_start(out=outpcol, in_=outpcol_ap)
    w1oT = const.tile([128, NDC, HE], F32)
    nc.sync.dma_start(out=w1oT, in_=w1oT_ap)
    w2colb = const.tile([128, NEC], BF16)
    nc.sync.dma_start(out=w2colb, in_=w2col_ap)
    # tail-only tensors: tiles declared now, loads deferred into batch 0's
    # pipeline so their DMA traffic doesn't contend with the first ctx blocks
    id128f = const.tile([128, 128], F32)
    onesv = const.tile([128, 130], F32)
    ones_row = onesv[:1, 0:128]
    ones128 = onesv[:, 128:129]
    ones1f = onesv[:1, 129:130]
    boutcol = const.tile([128, NEC], F32)
    woutT = const.tile([128, 8, HE], F32)

    def emit_tail_loads():
        nc.sync.dma_start(out=id128f, in_=ins["id128f"])
        nc.sync.dma_start(out=onesv, in_=ins["onesv"])
        nc.sync.dma_start(out=boutcol, in_=boutcol_ap)
        nc.sync.dma_start(out=woutT, in_=woutT_ap)

    # ---- prefetch batch 0's first ctx blocks ----
    ctx_bs = [
        ctx_ap[b].rearrange("(k p ss) d -> k p ss d", ss=NSS, p=128)
        for b in range(BC)
    ]

    def start_load(b, k):
        ct = ctx_pool.tile([128, NSS, HE], FP8, tag="ctx")
        nc.gpsimd.dma_start(out=ct, in_=ctx_bs[b][k])
        return ct
    preloaded = {k: start_load(0, k) for k in range(4)}

    # ---- per-batch tanh offsets: off[b] = W1[:, HE:] @ output_b ----
    # (emitted inside batch 0's pipeline, after the first ctx transposes, so
    # the DMA-semaphore-gated matmuls don't hold up the transposes in the
    # PE's static instruction order)
    offsb = const.tile([128, BC * NEC], F32)

    def emit_offs(bs):
        ps = psum_misc.tile([128, len(bs) * NEC], F32, tag="misc")
        for i, b in enumerate(bs):
            for ec in range(NEC):
                for dco in range(4):
                    nc.tensor.matmul(
                        ps[:, i * NEC + ec: i * NEC + ec + 1],
                        lhsT=w1oT[:, dco, ec * 128:(ec + 1) * 128],
                        rhs=outpcol[:, b * 4 + dco: b * 4 + dco + 1],
                        start=(dco == 0),
                        stop=(dco == 3),
                    )
        nc.scalar.copy(offsb[:, bs[0] * NEC:(bs[-1] + 1) * NEC], ps)

    # ---- per-batch final-tanh bias: bias2[b] = Wout[:, HE:] @ output_b + bout
    # (emitted after batch 0's blocks — only needed by the first tail) ----
    bias2sb = const.tile([128, BC * NEC], F32)

    def emit_bias2():
        ps2 = psum_misc.tile([128, BC * NEC], F32, tag="misc")
        for b in range(BC):
            for ec in range(NEC):
                for dco in range(4):
                    nc.tensor.matmul(
                        ps2[:, b * NEC + ec: b * NEC + ec + 1],
                        lhsT=woutT[:, 4 + dco, ec * 128:(ec + 1) * 128],
                        rhs=outpcol[:, b * 4 + dco: b * 4 + dco + 1],
                        start=(dco == 0),
                        stop=(dco == 3),
                    )
        for b in range(BC):
            nc.vector.tensor_add(
                bias2sb[:, b * NEC:(b + 1) * NEC],
                ps2[:, b * NEC:(b + 1) * NEC],
                boutcol,
            )

    # ---- main loop over batches (tails deferred one batch for overlap) ----
    def emit_blocks(b, pre_energy_hook=None):
        ctx_tiles = []
        pcol = psum_pcol.tile([128, SCHUNKS], F32)
        lg_pending = [None]

        def logit_mms(kp, tanh_p):
            for half in range(2):
                for ss in range(NSS):
                    j = (kp * 2 + half) * NSS + ss
                    for ec in range(NEC):
                        nc.tensor.matmul(
                            pcol[:, j:j + 1],
                            lhsT=tanh_p[ec][:, half, ss * 128:(ss + 1) * 128],
                            rhs=w2colb[:, ec:ec + 1],
                            start=(ec == 0),
                            stop=(ec == NEC - 1),
                        )

        def load_and_transpose(k):
            # load one s-block [128, 4, 512], casting fp32 -> fp8e4 in the DMA
            if b == 0 and k in preloaded:
                ct = preloaded.pop(k)
            else:
                ct = start_load(b, k)

            ctxT = []
            for dp in range(NDC // 2):
                pt = psum_tp.tile([128, 2, SBLK, 2], FP8, tag="tp")
                for half in range(2):
                    dc = dp * 2 + half
                    for ss in range(NSS):
                        nc.tensor.transpose(
                            pt[:, half, ss * 128:(ss + 1) * 128, 0],
                            ct[:, ss, dc * 128:(dc + 1) * 128],
                            id128q,
                        )
                st = ctxT_pool.tile([128, 2, SBLK, 2], FP8, tag="ctxT")
                nc.vector.tensor_copy(
                    st.bitcast(mybir.dt.uint16), pt.bitcast(mybir.dt.uint16)
                )
                ctxT.append(st[:, :, :, 0])
            return ct, ctxT

        # transpose stage runs one block ahead so the energy matmuls never
        # wait on the DVE copyback of their own block; energy psum + tanh are
        # fused over PAIRS of s-blocks (2-bank psum tile, one ACT op per pair)
        cur = load_and_transpose(0)
        ctxT_pair = [None, None]
        for k in range(NSBLK):
            ct, ctxT = cur
            ctx_tiles.append(ct)
            ctxT_pair[k % 2] = ctxT
            if k + 1 < NSBLK:
                cur = load_and_transpose(k + 1)
            if pre_energy_hook is not None and k in pre_energy_hook:
                pre_energy_hook[k]()
            if k % 2 == 0:
                continue
            kp = k // 2

            # energyT[e_chunk, s_blk] = sum_pair W1c8[pair,ec].T @ ctxT[pair]
            # (fp8 DoubleRow: each matmul contracts 2 d-chunks = 256 rows)
            tanh_p = []
            for ec in range(NEC):
                pe = psum_en.tile([128, 2, SBLK], F32, tag="en")
                for half in range(2):
                    for pair in range(NDC // 2):
                        nc.tensor.matmul(
                            pe[:, half, :],
                            lhsT=w1c8sb[pair][:, :, ec * 128:(ec + 1) * 128],
                            rhs=ctxT_pair[half][pair],
                            start=(pair == 0),
                            stop=(pair == NDC // 2 - 1),
                            perf_mode=DOUBLE_ROW,
                        )
                th = tanh_pool.tile([128, 2, SBLK], BF16, tag="tanh")
                nc.scalar.activation(
                    th, pe, AF.Tanh,
                    bias=offsb[:, b * NEC + ec: b * NEC + ec + 1],
                    scale=1.0 / W1SCALE,
                )
                tanh_p.append(th)

            # logit matvecs for the PREVIOUS pair, so the PE's static order
            # never waits on a tanh that ACT has only just been issued
            if lg_pending[0] is not None:
                logit_mms(*lg_pending[0])
            lg_pending[0] = (kp, tanh_p)
        logit_mms(*lg_pending[0])

        # exp right away (frees pcol for the next batch; tail only needs pexp)
        pexp = small.tile([128, SCHUNKS], BF16, tag="pexp")
        rowsum = small.tile([128, 1], F32, tag="rowsum")
        nc.scalar.activation(pexp, pcol, AF.Exp, accum_out=rowsum)
        return ctx_tiles, pexp, rowsum

    def emit_tail(b, ctx_tiles, pexp, rowsum):
        # normalization chain (1/Z broadcast column) ...
        pd = psum_misc.tile([1, 1], F32, tag="misc")
        nc.tensor.matmul(pd, lhsT=rowsum, rhs=ones128)
        inv = small.tile([1, 1], F32, tag="inv")
        nc.vector.reciprocal(inv, pd)
        pinvb = psum_misc.tile([128, 1], F32, tag="misc")
        nc.tensor.matmul(pinvb, lhsT=ones_row, rhs=inv)
        invb = small.tile([128, 1], F32, tag="invb")
        nc.vector.tensor_copy(invb, pinvb)

        # ... runs in parallel with the unnormalized mix -> Wout matmuls:
        # mix columns directly: ctx chunks as stationary (LDW stream),
        # exp-weight column as the 1-wide moving operand; accumulate over j
        pmcol = psum_misc.tile([128, 4], F32, tag="misc")
        for dc in range(4):
            for j in range(SCHUNKS):
                nc.tensor.matmul(
                    pmcol[:, dc:dc + 1],
                    lhsT=ctx_tiles[j // NSS][:, j % NSS, dc * 128:(dc + 1) * 128],
                    rhs=pexp[:, j:j + 1],
                    start=(j == 0),
                    stop=(j == SCHUNKS - 1),
                )
        mc = small.tile([128, 4], F32, tag="mc_sb")
        nc.vector.tensor_copy(mc, pmcol)

        # final: out_col[ec] = tanh(invZ * (WoutT[:, :4] @ mc) + bias2[b])
        pfo = psum_misc.tile([128, NEC], F32, tag="misc")
        for ec in range(NEC):
            for dc in range(4):
                nc.tensor.matmul(
                    pfo[:, ec:ec + 1],
                    lhsT=woutT[:, dc, ec * 128:(ec + 1) * 128],
                    rhs=mc[:, dc:dc + 1],
                    start=(dc == 0),
                    stop=(dc == 3),
                )
        fo = small.tile([128, NEC], F32, tag="fo_sb")
        for ec in range(NEC):
            nc.scalar.activation(
                fo[:, ec:ec + 1], pfo[:, ec:ec + 1], AF.Tanh,
                bias=bias2sb[:, b * NEC + ec: b * NEC + ec + 1],
                scale=invb,
            )

        # back to a row [1, 512] and out
        por = psum_misc.tile([1, HE], F32, tag="misc")
        for ec in range(NEC):
            nc.tensor.transpose(
                por[:, ec * 128:(ec + 1) * 128], fo[:, ec:ec + 1], id128f
            )
        orow = small.tile([1, HE], F32, tag="orow")
        nc.vector.tensor_copy(orow, por)
        nc.sync.dma_start(out=out_ap[b], in_=orow)

    def emit_late_setup():
        emit_tail_loads()
        emit_offs([1, 2, 3])
        emit_bias2()

    hooks0 = {1: lambda: emit_offs([0]), 5: emit_late_setup}
    pending = None
    for b in range(BC):
        state = emit_blocks(b, pre_energy_hook=hooks0 if b == 0 else None)
        if pending is not None:
            emit_tail(pending[0], *pending[1])
        pending = (b, state)
    emit_tail(pending[0], *pending[1])


INPUT_SPECS = {
    "context": ((BC, S, HE), F32),
    "w1c8": ((2, 128, 2, HE), FP8),
    "w1oT": ((128, NDC, HE), F32),
    "woutT": ((128, 8, HE), F32),
    "w2col": ((128, NEC), BF16),
    "boutcol": ((128, NEC), F32),
    "outpcol": ((128, BC * 4), F32),
    "id128q": ((128, 128), FP8),
    "id128f": ((128, 128), F32),
    "onesv": ((128, 130), F32),
}

_CACHE = {}


def build_nc():
    if "nc" in _CACHE:
        return _CACHE["nc"]
    nc = bacc.Bacc("TRN2", target_bir_lowering=False, debug=False,
                   num_devices=NCORES)
    ins = {
        name: nc.dram_tensor(name, list(shape), dt, kind="ExternalInput").ap()
        for name, (shape, dt) in INPUT_SPECS.items()
    }
    out = nc.dram_tensor("out", [BC, 1, HE], F32, kind="ExternalOutput").ap()
    with tile.TileContext(nc) as tc:
        attention_kernel(tc, out, ins)
    nc.compile()
    _CACHE["nc"] = nc
    return nc


def make_in_maps(output, context, W1, W2, Wout, bout):
    """Shard the activations; precompute all weight-only layout transforms
    host-side (transposes, fp8 scaling + DoubleRow pairing, columnization)."""
    f8 = mybir.dt.np(FP8)
    bf = mybir.dt.np(BF16)
    W1 = np.asarray(W1, np.float32)
    Wout = np.asarray(Wout, np.float32)
    # W1 context half, transposed [d, e], scaled, DR-paired [pair, k, half, e]
    w1cT = W1[:, :HE].T
    w1c8 = np.clip(w1cT * W1SCALE, -240.0, 240.0).reshape(2, 2, 128, HE)
    w1c8 = np.ascontiguousarray(w1c8.transpose(0, 2, 1, 3)).astype(f8)
    # W1 output half, transposed [k, dco, e]
    w1oT = np.ascontiguousarray(W1[:, HE:].T.reshape(NDC, 128, HE)
                                .transpose(1, 0, 2))
    # Wout transposed [k, dc, e]
    woutT = np.ascontiguousarray(Wout.T.reshape(8, 128, HE).transpose(1, 0, 2))
    w2col = np.ascontiguousarray(np.asarray(W2, np.float32)[0]
                                 .reshape(NEC, 128).T).astype(bf)
    boutcol = np.ascontiguousarray(np.asarray(bout, np.float32)
                                   .reshape(NEC, 128).T)
    id128f = np.eye(128, dtype=np.float32)
    id128q = id128f.astype(f8)
    onesv = np.ones((128, 130), np.float32)

    # output rows columnized per core: outpcol[k, b*4+dc] = output[b,0,dc*128+k]
    outp = np.asarray(output, np.float32).reshape(B, 4, 128)
    maps = []
    for i in range(NCORES):
        sl = slice(i * BC, (i + 1) * BC)
        outpcol = np.ascontiguousarray(
            outp[sl].reshape(BC * 4, 128).T)
        maps.append({
            "context": np.ascontiguousarray(context[sl], dtype=np.float32),
            "w1c8": w1c8,
            "w1oT": w1oT,
            "woutT": woutT,
            "w2col": w2col,
            "boutcol": boutcol,
            "outpcol": outpcol,
            "id128q": id128q,
            "id128f": id128f,
            "onesv": onesv,
        })
    return maps


def run(inputs, trace=False):
    from concourse.bass_utils import run_bass_kernel_spmd

    nc = build_nc()
    in_maps = make_in_maps(**inputs)
    res = run_bass_kernel_spmd(nc, in_maps, list(range(NCORES)), trace=trace)
    out = np.concatenate([res.results[i]["out"] for i in range(NCORES)], axis=0)
    return out, res


def kernel(output, context, W1, W2, Wout, bout):
    out, _ = run(dict(output=output, context=context, W1=W1, W2=W2,
                      Wout=Wout, bout=bout))
    return out


# revision 52
# speedup vs baseline: 1.2408x; 1.2408x over previous
"""Trainium2 Bass kernel for nn_Attention_76089640616322.

Bahdanau-style attention:
  B, S, HE, DOUT = 32, 4096, 512, 512  (HD = 1024)
  energy = tanh(concat([context, broadcast(output)], -1) @ W1.T)   [B,S,HE]
  attn   = softmax(energy @ W2.T, axis=S)                           [B,1,S]
  mix    = attn @ context                                           [B,1,HE]
  out    = tanh(concat([mix, output], -1) @ Wout.T + bout)          [B,1,HE]

Sharding: pure data parallel, batch dim across 8 cores (4 batches/core),
weights replicated.

Structure (per core, context batch kept SBUF-resident in fp8):
- All weight-only layout transforms (W1/Wout transposes, fp8 scaling and
  DoubleRow pairing of W1's context half, W2/bout columnization) are done
  host-side in make_in_maps; the device loads them in their final layout.
- The broadcast `output` columns of the concat make W1 @ concat(...) =
  W1[:, :HE] @ context + (W1[:, HE:] @ output_b); the second term is a
  per-batch constant fused as a per-partition bias into the tanh (ACT).
- context is cast fp32->fp8e4 during the HBM load (SWDGE cast DMA) with a
  per-partition-contiguous rearrange (s = k*512 + 4p + ss) so each DMA
  descriptor covers an 8 KiB HBM-side run. The fp8 copy serves both the
  energy matmul and the mix contraction (mix contributes only ~2% of the
  final pre-tanh signal, so fp8 there is harmless; measured 4.3e-4 rel
  err end-to-end in numpy).
- ctx is PE-transposed in fp8 (1 cycle/row). The PE writes fp8 transpose
  results 2 bytes apart at 4-byte aligned starts (HW rules), so the
  psum/sbuf tiles carry a trailing stride-2 dim; the PSUM->SBUF copyback
  moves value+pad byte pairs as uint16 (keeps the DVE 2-byte 2X mode) and
  the DoubleRow rhs reads the fp8 values through the stride-2 view.
- energyT = W1cT.T @ ctxT runs as fp8 DoubleRow matmuls (2 k-tiles per
  instruction, 2 MACs/cell/cycle) with fp32 PSUM accumulation. W1cT is
  pre-scaled by 64 into fp8 (keeps the sigma=0.02 weights out of the fp8
  denormal range); the tanh ACT applies the compensating 1/64 scale.
- energy psum + tanh are fused over PAIRS of s-blocks (2-bank psum tile,
  one ACT op per pair) to amortize the ACT per-op PSUM-access overhead.
- The W2 logit dot and the softmax-weighted mix contraction both use the
  "stationary flip": the large tile (tanhT / ctx chunk) rides the
  weight-load stream and the 1-wide vector is the moving operand, so their
  outputs land directly in column form and cost no 512-cycle PE streams.
- Softmax is computed unnormalized (|logit| <= ||W2||_1 ~ 8, exp safe in
  fp32; exp on ACT with a free per-partition accum for the denominator);
  the 1/Z and the Wout output-half contribution (+bout) are both folded
  into the final tanh via its scale/bias operands, so the reciprocal chain
  runs in parallel with the mix->Wout matmuls.
- Batch tails are deferred one batch so their serial chain overlaps the
  next batch's block pipeline.
"""

from contextlib import ExitStack

import numpy as np

import concourse.bass as bass
import concourse.tile as tile
from concourse import bacc, mybir
from concourse._compat import with_exitstack
from concourse.masks import make_identity

B, S, HE, DOUT = 32, 4096, 512, 512
HD = HE + DOUT
NCORES = 8
BC = B // NCORES  # batches per core

F32 = mybir.dt.float32
BF16 = mybir.dt.bfloat16
FP8 = mybir.dt.float8e4
AF = mybir.ActivationFunctionType
DOUBLE_ROW = mybir.MatmulPerfMode.DoubleRow
W1SCALE = 64.0

NSBLK = 8       # s-blocks per batch (512 s each)
SBLK = S // NSBLK   # 512
NSS = SBLK // 128   # 4 subtiles of 128 s per block
NEC = HE // 128     # 4 e-chunks
NDC = HE // 128     # 4 d-chunks for the context half of W1
SCHUNKS = S // 128  # 32 s-chunks of 128 per batch


@with_exitstack
def attention_kernel(ctx: ExitStack, tc: tile.TileContext, out_ap, ins):
    nc = tc.nc

    ctx_ap = ins["context"]    # [BC, S, HE] f32
    # packed weights/constants (see make_in_maps for the layouts):
    blobA_ap = ins["blobA"]    # id128q, w1c8 pair0/1, outpcol
    blobB_ap = ins["blobB"]    # w1oT, w2col
    blobC_ap = ins["blobC"]    # id128f, onesv, boutcol, woutT (tail-only)

    const = ctx.enter_context(tc.tile_pool(name="const", bufs=1))
    ctx_pool = ctx.enter_context(tc.tile_pool(name="ctx", bufs=18))
    ctxT_pool = ctx.enter_context(tc.tile_pool(name="ctxT", bufs=8))
    tanh_pool = ctx.enter_context(tc.tile_pool(name="tanh", bufs=8))
    small = ctx.enter_context(tc.tile_pool(name="small", bufs=2))

    psum_tp = ctx.enter_context(tc.tile_pool(name="ptp", bufs=2, space="PSUM"))
    psum_en = ctx.enter_context(tc.tile_pool(name="pen", bufs=2, space="PSUM"))
    psum_pcol = ctx.enter_context(tc.tile_pool(name="ppcol", bufs=1, space="PSUM"))
    psum_misc = ctx.enter_context(tc.tile_pool(name="pmisc", bufs=1, space="PSUM"))

    # ---- weight/constant loads: one HWDGE DMA per blob, ordered by first
    # use (blobA gates batch 0's transposes/energy; blobC is tail-only and
    # deferred into batch 0's pipeline to stay off the early DMA path) ----
    U8 = mybir.dt.uint8
    blobA = const.tile([128, 2208], U8)
    nc.sync.dma_start(out=blobA, in_=blobA_ap)
    id128q = blobA[:, 0:128].bitcast(FP8)
    w1c8sb = [
        blobA[:, 128:1152].bitcast(FP8).rearrange("p (a e) -> p a e", a=2),
        blobA[:, 1152:2176].bitcast(FP8).rearrange("p (a e) -> p a e", a=2),
    ]
    outpcol = blobA[:, 2176:2208].bitcast(BF16)

    blobB = const.tile([128, 4104], U8)
    nc.sync.dma_start(out=blobB, in_=blobB_ap)
    w1oT = blobB[:, 0:4096].bitcast(BF16).rearrange("p (c d) -> p c d", c=NDC)
    w2colb = blobB[:, 4096:4104].bitcast(BF16)

    blobC = const.tile([128, 9240], U8)
    id128f = blobC[:, 0:512].bitcast(F32)
    onesv = blobC[:, 512:1032].bitcast(F32)
    ones_row = onesv[:1, 0:128]
    ones128 = onesv[:, 128:129]
    ones1f = onesv[:1, 129:130]
    boutcol = blobC[:, 1032:1048].bitcast(F32)
    woutT = blobC[:, 1048:9240].bitcast(BF16).rearrange("p (c d) -> p c d", c=8)

    def emit_tail_loads():
        # SWDGE so the transfer queues behind batch 0's ctx loads instead of
        # hogging the DMA pipe while the first blocks are still arriving
        nc.gpsimd.dma_start(out=blobC, in_=blobC_ap)

    # ---- prefetch batch 0's first ctx blocks ----
    ctx_bs = [
        ctx_ap[b].rearrange("(k p ss) d -> k p ss d", ss=NSS, p=128)
        for b in range(BC)
    ]

    def start_load(b, k):
        ct = ctx_pool.tile([128, NSS, HE], FP8, tag="ctx")
        nc.gpsimd.dma_start(out=ct, in_=ctx_bs[b][k])
        return ct
    preloaded = {k: start_load(0, k) for k in range(4)}

    # ---- per-batch tanh offsets: off[b] = W1[:, HE:] @ output_b ----
    # (emitted inside batch 0's pipeline, after the first ctx transposes, so
    # the DMA-semaphore-gated matmuls don't hold up the transposes in the
    # PE's static instruction order)
    offsb = const.tile([128, BC * NEC], F32)

    def emit_offs(bs):
        ps = psum_misc.tile([128, len(bs) * NEC], F32, tag="misc")
        for i, b in enumerate(bs):
            for ec in range(NEC):
                for dco in range(4):
                    nc.tensor.matmul(
                        ps[:, i * NEC + ec: i * NEC + ec + 1],
                        lhsT=w1oT[:, dco, ec * 128:(ec + 1) * 128],
                        rhs=outpcol[:, b * 4 + dco: b * 4 + dco + 1],
                        start=(dco == 0),
                        stop=(dco == 3),
                    )
        nc.scalar.copy(offsb[:, bs[0] * NEC:(bs[-1] + 1) * NEC], ps)

    # ---- per-batch final-tanh bias: bias2[b] = Wout[:, HE:] @ output_b + bout
    # (emitted after batch 0's blocks — only needed by the first tail) ----
    bias2sb = const.tile([128, BC * NEC], F32)

    def emit_bias2():
        ps2 = psum_misc.tile([128, BC * NEC], F32, tag="misc")
        for b in range(BC):
            for ec in range(NEC):
                for dco in range(4):
                    nc.tensor.matmul(
                        ps2[:, b * NEC + ec: b * NEC + ec + 1],
                        lhsT=woutT[:, 4 + dco, ec * 128:(ec + 1) * 128],
                        rhs=outpcol[:, b * 4 + dco: b * 4 + dco + 1],
                        start=(dco == 0),
                        stop=(dco == 3),
                    )
        for b in range(BC):
            nc.vector.tensor_add(
                bias2sb[:, b * NEC:(b + 1) * NEC],
                ps2[:, b * NEC:(b + 1) * NEC],
                boutcol,
            )

    # ---- main loop over batches (tails deferred one batch for overlap) ----
    def emit_blocks(b, pre_energy_hook=None):
        ctx_tiles = []
        pcol = psum_pcol.tile([128, SCHUNKS], F32)
        lg_pending = [None]

        def logit_mms(kp, tanh_p):
            for half in range(2):
                for ss in range(NSS):
                    j = (kp * 2 + half) * NSS + ss
                    for ec in range(NEC):
                        nc.tensor.matmul(
                            pcol[:, j:j + 1],
                            lhsT=tanh_p[ec][:, half, ss * 128:(ss + 1) * 128],
                            rhs=w2colb[:, ec:ec + 1],
                            start=(ec == 0),
                            stop=(ec == NEC - 1),
                        )

        def load_and_transpose(k):
            # load one s-block [128, 4, 512], casting fp32 -> fp8e4 in the DMA
            if b == 0 and k in preloaded:
                ct = preloaded.pop(k)
            else:
                ct = start_load(b, k)

            ctxT = []
            for dp in range(NDC // 2):
                pt = psum_tp.tile([128, 2, SBLK, 2], FP8, tag="tp")
                for half in range(2):
                    dc = dp * 2 + half
                    for ss in range(NSS):
                        nc.tensor.transpose(
                            pt[:, half, ss * 128:(ss + 1) * 128, 0],
                            ct[:, ss, dc * 128:(dc + 1) * 128],
                            id128q,
                        )
                st = ctxT_pool.tile([128, 2, SBLK, 2], FP8, tag="ctxT")
                nc.vector.tensor_copy(
                    st.bitcast(mybir.dt.uint16), pt.bitcast(mybir.dt.uint16)
                )
                ctxT.append(st[:, :, :, 0])
            return ct, ctxT

        # transpose stage runs one block ahead so the energy matmuls never
        # wait on the DVE copyback of their own block; energy psum + tanh are
        # fused over PAIRS of s-blocks (2-bank psum tile, one ACT op per pair)
        cur = load_and_transpose(0)
        ctxT_pair = [None, None]
        for k in range(NSBLK):
            ct, ctxT = cur
            ctx_tiles.append(ct)
            ctxT_pair[k % 2] = ctxT
            if k % 2 == 0:
                # next (odd) block's transposes go ahead of the pair's energy
                # so the copyback of THIS block is hidden behind PE work
                cur = load_and_transpose(k + 1)
                if pre_energy_hook is not None and k in pre_energy_hook:
                    pre_energy_hook[k]()
                continue
            kp = k // 2

            # energyT[e_chunk, s_blk] = sum_pair W1c8[pair,ec].T @ ctxT[pair]
            # (fp8 DoubleRow: each matmul contracts 2 d-chunks = 256 rows)
            tanh_p = []
            for ec in range(NEC):
                pe = psum_en.tile([128, 2, SBLK], F32, tag="en")
                for half in range(2):
                    for pair in range(NDC // 2):
                        nc.tensor.matmul(
                            pe[:, half, :],
                            lhsT=w1c8sb[pair][:, :, ec * 128:(ec + 1) * 128],
                            rhs=ctxT_pair[half][pair],
                            start=(pair == 0),
                            stop=(pair == NDC // 2 - 1),
                            perf_mode=DOUBLE_ROW,
                        )
                th = tanh_pool.tile([128, 2, SBLK], BF16, tag="tanh")
                nc.scalar.activation(
                    th, pe, AF.Tanh,
                    bias=offsb[:, b * NEC + ec: b * NEC + ec + 1],
                    scale=1.0 / W1SCALE,
                )
                tanh_p.append(th)

            # logit matvecs for the PREVIOUS pair, so the PE's static order
            # never waits on a tanh that ACT has only just been issued
            if lg_pending[0] is not None:
                logit_mms(*lg_pending[0])
            lg_pending[0] = (kp, tanh_p)
            # the next (even) block's load+transposes ride after the energy
            if k + 1 < NSBLK:
                cur = load_and_transpose(k + 1)
            if pre_energy_hook is not None and k in pre_energy_hook:
                pre_energy_hook[k]()
        logit_mms(*lg_pending[0])

        # exp right away (frees pcol for the next batch; tail only needs pexp)
        pexp = small.tile([128, SCHUNKS], BF16, tag="pexp")
        rowsum = small.tile([128, 1], F32, tag="rowsum")
        nc.scalar.activation(pexp, pcol, AF.Exp, accum_out=rowsum)
        return ctx_tiles, pexp, rowsum

    def emit_tail(b, ctx_tiles, pexp, rowsum):
        # normalization chain (1/Z broadcast column) ...
        pd = psum_misc.tile([1, 1], F32, tag="misc")
        nc.tensor.matmul(pd, lhsT=rowsum, rhs=ones128)
        inv = small.tile([1, 1], F32, tag="inv")
        nc.vector.reciprocal(inv, pd)
        pinvb = psum_misc.tile([128, 1], F32, tag="misc")
        nc.tensor.matmul(pinvb, lhsT=ones_row, rhs=inv)
        invb = small.tile([128, 1], F32, tag="invb")
        nc.vector.tensor_copy(invb, pinvb)

        # ... runs in parallel with the unnormalized mix -> Wout matmuls:
        # mix columns directly: ctx chunks as stationary (LDW stream),
        # exp-weight column as the 1-wide moving operand; accumulate over j
        pmcol = psum_misc.tile([128, 4], F32, tag="misc")
        for dc in range(4):
            for j in range(SCHUNKS):
                nc.tensor.matmul(
                    pmcol[:, dc:dc + 1],
                    lhsT=ctx_tiles[j // NSS][:, j % NSS, dc * 128:(dc + 1) * 128],
                    rhs=pexp[:, j:j + 1],
                    start=(j == 0),
                    stop=(j == SCHUNKS - 1),
                )
        mc = small.tile([128, 4], BF16, tag="mc_sb")
        nc.vector.tensor_copy(mc, pmcol)

        # final: out_col[ec] = tanh(invZ * (WoutT[:, :4] @ mc) + bias2[b])
        pfo = psum_misc.tile([128, NEC], F32, tag="misc")
        for ec in range(NEC):
            for dc in range(4):
                nc.tensor.matmul(
                    pfo[:, ec:ec + 1],
                    lhsT=woutT[:, dc, ec * 128:(ec + 1) * 128],
                    rhs=mc[:, dc:dc + 1],
                    start=(dc == 0),
                    stop=(dc == 3),
                )
        fo = small.tile([128, NEC], F32, tag="fo_sb")
        for ec in range(NEC):
            nc.scalar.activation(
                fo[:, ec:ec + 1], pfo[:, ec:ec + 1], AF.Tanh,
                bias=bias2sb[:, b * NEC + ec: b * NEC + ec + 1],
                scale=invb,
            )

        # back to a row [1, 512] and out
        por = psum_misc.tile([1, HE], F32, tag="misc")
        for ec in range(NEC):
            nc.tensor.transpose(
                por[:, ec * 128:(ec + 1) * 128], fo[:, ec:ec + 1], id128f
            )
        orow = small.tile([1, HE], F32, tag="orow")
        nc.vector.tensor_copy(orow, por)
        nc.sync.dma_start(out=out_ap[b], in_=orow)

    def emit_late_setup():
        emit_tail_loads()
        emit_offs([1, 2, 3])
        emit_bias2()

    hooks0 = {1: lambda: emit_offs([0]), 5: emit_late_setup}
    pending = None
    for b in range(BC):
        state = emit_blocks(b, pre_energy_hook=hooks0 if b == 0 else None)
        if pending is not None:
            emit_tail(pending[0], *pending[1])
        pending = (b, state)
    emit_tail(pending[0], *pending[1])


INPUT_SPECS = {
    "context": ((BC, S, HE), F32),
    "blobA": ((128, 2208), mybir.dt.uint8),
    "blobB": ((128, 4104), mybir.dt.uint8),
    "blobC": ((128, 9240), mybir.dt.uint8),
}

_CACHE = {}


def build_nc():
    if "nc" in _CACHE:
        return _CACHE["nc"]
    nc = bacc.Bacc("TRN2", target_bir_lowering=False, debug=False,
                   num_devices=NCORES)
    ins = {
        name: nc.dram_tensor(name, list(shape), dt, kind="ExternalInput").ap()
        for name, (shape, dt) in INPUT_SPECS.items()
    }
    out = nc.dram_tensor("out", [BC, 1, HE], F32, kind="ExternalOutput").ap()
    with tile.TileContext(nc) as tc:
        attention_kernel(tc, out, ins)
    nc.compile()
    _CACHE["nc"] = nc
    return nc


def _u8(a):
    return np.ascontiguousarray(a).reshape(128, -1).view(np.uint8)


def make_in_maps(output, context, W1, W2, Wout, bout):
    """Shard the activations; precompute all weight-only layout transforms
    host-side (transposes, fp8 scaling + DoubleRow pairing, columnization)
    and pack them into three per-partition byte blobs (one DMA each)."""
    f8 = mybir.dt.np(FP8)
    bf = mybir.dt.np(BF16)
    W1 = np.asarray(W1, np.float32)
    Wout = np.asarray(Wout, np.float32)
    # W1 context half, transposed [d, e], scaled, DR-paired [pair, k, half, e]
    w1cT = W1[:, :HE].T
    w1c8 = np.clip(w1cT * W1SCALE, -240.0, 240.0).reshape(2, 2, 128, HE)
    w1c8 = np.ascontiguousarray(w1c8.transpose(0, 2, 1, 3)).astype(f8)
    # W1 output half, transposed [k, dco, e], bf16
    w1oT = np.ascontiguousarray(W1[:, HE:].T.reshape(NDC, 128, HE)
                                .transpose(1, 0, 2)).astype(bf)
    # Wout transposed [k, dc, e], bf16
    woutT = np.ascontiguousarray(Wout.T.reshape(8, 128, HE)
                                 .transpose(1, 0, 2)).astype(bf)
    w2col = np.ascontiguousarray(np.asarray(W2, np.float32)[0]
                                 .reshape(NEC, 128).T).astype(bf)
    boutcol = np.ascontiguousarray(np.asarray(bout, np.float32)
                                   .reshape(NEC, 128).T)
    id128f = np.eye(128, dtype=np.float32)
    id128q = id128f.astype(f8)
    onesv = np.ones((128, 130), np.float32)

    blobB = np.concatenate([_u8(w1oT), _u8(w2col)], axis=1)
    blobC = np.concatenate(
        [_u8(id128f), _u8(onesv), _u8(boutcol), _u8(woutT)], axis=1)

    # output rows columnized per core: outpcol[k, b*4+dc] = output[b,0,dc*128+k]
    outp = np.asarray(output, np.float32).reshape(B, 4, 128)
    maps = []
    for i in range(NCORES):
        sl = slice(i * BC, (i + 1) * BC)
        outpcol = np.ascontiguousarray(
            outp[sl].reshape(BC * 4, 128).T).astype(bf)
        blobA = np.concatenate(
            [_u8(id128q), _u8(w1c8[0]), _u8(w1c8[1]), _u8(outpcol)], axis=1)
        maps.append({
            "context": np.ascontiguousarray(context[sl], dtype=np.float32),
            "blobA": blobA,
            "blobB": blobB,
            "blobC": blobC,
        })
    return maps


def run(inputs, trace=False):
    from concourse.bass_utils import run_bass_kernel_spmd

    nc = build_nc()
    in_maps = make_in_maps(**inputs)
    res = run_bass_kernel_spmd(nc, in_maps, list(range(NCORES)), trace=trace)
    out = np.concatenate([res.results[i]["out"] for i in range(NCORES)], axis=0)
    return out, res


def kernel(output, context, W1, W2, Wout, bout):
    out, _ = run(dict(output=output, context=context, W1=W1, W2=W2,
                      Wout=Wout, bout=bout))
    return out


# revision 53
# speedup vs baseline: 1.3042x; 1.0511x over previous
"""Trainium2 Bass kernel for nn_Attention_76089640616322.

Bahdanau-style attention:
  B, S, HE, DOUT = 32, 4096, 512, 512  (HD = 1024)
  energy = tanh(concat([context, broadcast(output)], -1) @ W1.T)   [B,S,HE]
  attn   = softmax(energy @ W2.T, axis=S)                           [B,1,S]
  mix    = attn @ context                                           [B,1,HE]
  out    = tanh(concat([mix, output], -1) @ Wout.T + bout)          [B,1,HE]

Sharding: pure data parallel, batch dim across 8 cores (4 batches/core),
weights replicated.

Structure (per core, context batch kept SBUF-resident in fp8):
- All weight-only layout transforms (W1/Wout transposes, fp8 scaling and
  DoubleRow pairing of W1's context half, W2/bout columnization) are done
  host-side in make_in_maps; the device loads them in their final layout.
- The broadcast `output` columns of the concat make W1 @ concat(...) =
  W1[:, :HE] @ context + (W1[:, HE:] @ output_b); the second term is a
  per-batch constant fused as a per-partition bias into the tanh (ACT).
- context is cast fp32->fp8e4 during the HBM load (SWDGE cast DMA) with a
  per-partition-contiguous rearrange (s = k*512 + 4p + ss) so each DMA
  descriptor covers an 8 KiB HBM-side run. The fp8 copy serves both the
  energy matmul and the mix contraction (mix contributes only ~2% of the
  final pre-tanh signal, so fp8 there is harmless; measured 4.3e-4 rel
  err end-to-end in numpy).
- ctx is PE-transposed in fp8 (1 cycle/row). The PE writes fp8 transpose
  results 2 bytes apart at 4-byte aligned starts (HW rules), so the
  psum/sbuf tiles carry a trailing stride-2 dim; the PSUM->SBUF copyback
  moves value+pad byte pairs as uint16 (keeps the DVE 2-byte 2X mode) and
  the DoubleRow rhs reads the fp8 values through the stride-2 view.
- energyT = W1cT.T @ ctxT runs as fp8 DoubleRow matmuls (2 k-tiles per
  instruction, 2 MACs/cell/cycle) with fp32 PSUM accumulation. W1cT is
  pre-scaled by 64 into fp8 (keeps the sigma=0.02 weights out of the fp8
  denormal range); the tanh ACT applies the compensating 1/64 scale.
- energy psum + tanh are fused over PAIRS of s-blocks (2-bank psum tile,
  one ACT op per pair) to amortize the ACT per-op PSUM-access overhead.
- The W2 logit dot and the softmax-weighted mix contraction both use the
  "stationary flip": the large tile (tanhT / ctx chunk) rides the
  weight-load stream and the 1-wide vector is the moving operand, so their
  outputs land directly in column form and cost no 512-cycle PE streams.
- Softmax is computed unnormalized (|logit| <= ||W2||_1 ~ 8, exp safe in
  fp32; exp on ACT with a free per-partition accum for the denominator);
  the 1/Z and the Wout output-half contribution (+bout) are both folded
  into the final tanh via its scale/bias operands, so the reciprocal chain
  runs in parallel with the mix->Wout matmuls.
- Batch tails are deferred one batch so their serial chain overlaps the
  next batch's block pipeline.
"""

from contextlib import ExitStack

import numpy as np

import concourse.bass as bass
import concourse.tile as tile
from concourse import bacc, mybir
from concourse._compat import with_exitstack
from concourse.masks import make_identity

B, S, HE, DOUT = 32, 4096, 512, 512
HD = HE + DOUT
NCORES = 8
BC = B // NCORES  # batches per core

F32 = mybir.dt.float32
BF16 = mybir.dt.bfloat16
FP8 = mybir.dt.float8e4
AF = mybir.ActivationFunctionType
DOUBLE_ROW = mybir.MatmulPerfMode.DoubleRow
W1SCALE = 64.0

NSBLK = 8       # s-blocks per batch (512 s each)
SBLK = S // NSBLK   # 512
NSS = SBLK // 128   # 4 subtiles of 128 s per block
NEC = HE // 128     # 4 e-chunks
NDC = HE // 128     # 4 d-chunks for the context half of W1
SCHUNKS = S // 128  # 32 s-chunks of 128 per batch


@with_exitstack
def attention_kernel(ctx: ExitStack, tc: tile.TileContext, out_ap, ins):
    nc = tc.nc

    ctx_ap = ins["context"]    # [BC, S, HE] f32
    # packed weights/constants (see make_in_maps for the layouts):
    blobA_ap = ins["blobA"]    # id128q, w1c8 pair0/1, outpcol
    blobB_ap = ins["blobB"]    # w1oT, w2col
    blobC_ap = ins["blobC"]    # id128f, onesv, boutcol, woutT (tail-only)

    const = ctx.enter_context(tc.tile_pool(name="const", bufs=1))
    ctx_pool = ctx.enter_context(tc.tile_pool(name="ctx", bufs=26))
    ctxT_pool = ctx.enter_context(tc.tile_pool(name="ctxT", bufs=8))
    tanh_pool = ctx.enter_context(tc.tile_pool(name="tanh", bufs=8))
    small = ctx.enter_context(tc.tile_pool(name="small", bufs=2))

    psum_tp = ctx.enter_context(tc.tile_pool(name="ptp", bufs=2, space="PSUM"))
    psum_en = ctx.enter_context(tc.tile_pool(name="pen", bufs=2, space="PSUM"))
    psum_pcol = ctx.enter_context(tc.tile_pool(name="ppcol", bufs=1, space="PSUM"))
    psum_misc = ctx.enter_context(tc.tile_pool(name="pmisc", bufs=1, space="PSUM"))

    # ---- weight/constant loads: one HWDGE DMA per blob, ordered by first
    # use (blobA gates batch 0's transposes/energy; blobC is tail-only and
    # deferred into batch 0's pipeline to stay off the early DMA path) ----
    U8 = mybir.dt.uint8
    blobA = const.tile([128, 2208], U8)
    nc.sync.dma_start(out=blobA, in_=blobA_ap)
    id128q = blobA[:, 0:128].bitcast(FP8)
    w1c8sb = [
        blobA[:, 128:1152].bitcast(FP8).rearrange("p (a e) -> p a e", a=2),
        blobA[:, 1152:2176].bitcast(FP8).rearrange("p (a e) -> p a e", a=2),
    ]
    outpcol = blobA[:, 2176:2208].bitcast(BF16)

    blobB = const.tile([128, 4104], U8)
    nc.sync.dma_start(out=blobB, in_=blobB_ap)
    w1oT = blobB[:, 0:4096].bitcast(BF16).rearrange("p (c d) -> p c d", c=NDC)
    w2colb = blobB[:, 4096:4104].bitcast(BF16)

    blobC = const.tile([128, 9240], U8)
    id128f = blobC[:, 0:512].bitcast(F32)
    onesv = blobC[:, 512:1032].bitcast(F32)
    ones_row = onesv[:1, 0:128]
    ones128 = onesv[:, 128:129]
    ones1f = onesv[:1, 129:130]
    boutcol = blobC[:, 1032:1048].bitcast(F32)
    woutT = blobC[:, 1048:9240].bitcast(BF16).rearrange("p (c d) -> p c d", c=8)

    def emit_tail_loads():
        # SWDGE so the transfer queues behind batch 0's ctx loads instead of
        # hogging the DMA pipe while the first blocks are still arriving
        nc.gpsimd.dma_start(out=blobC, in_=blobC_ap)

    # ---- prefetch batch 0's first ctx blocks ----
    ctx_bs = [
        ctx_ap[b].rearrange("(k p ss) d -> k p ss d", ss=NSS, p=128)
        for b in range(BC)
    ]

    def start_load(b, k):
        ct = ctx_pool.tile([128, NSS, HE], FP8, tag="ctx")
        nc.gpsimd.dma_start(out=ct, in_=ctx_bs[b][k])
        return ct
    preloaded = {k: start_load(0, k) for k in range(4)}

    # ---- per-batch tanh offsets: off[b] = W1[:, HE:] @ output_b ----
    # (emitted inside batch 0's pipeline, after the first ctx transposes, so
    # the DMA-semaphore-gated matmuls don't hold up the transposes in the
    # PE's static instruction order)
    offsb = const.tile([128, BC * NEC], F32)

    def emit_offs(bs):
        ps = psum_misc.tile([128, len(bs) * NEC], F32, tag="misc")
        for i, b in enumerate(bs):
            for ec in range(NEC):
                for dco in range(4):
                    nc.tensor.matmul(
                        ps[:, i * NEC + ec: i * NEC + ec + 1],
                        lhsT=w1oT[:, dco, ec * 128:(ec + 1) * 128],
                        rhs=outpcol[:, b * 4 + dco: b * 4 + dco + 1],
                        start=(dco == 0),
                        stop=(dco == 3),
                    )
        nc.scalar.copy(offsb[:, bs[0] * NEC:(bs[-1] + 1) * NEC], ps)

    # ---- per-batch final-tanh bias: bias2[b] = Wout[:, HE:] @ output_b + bout
    # (emitted after batch 0's blocks — only needed by the first tail) ----
    bias2sb = const.tile([128, BC * NEC], F32)

    def emit_bias2():
        ps2 = psum_misc.tile([128, BC * NEC], F32, tag="misc")
        for b in range(BC):
            for ec in range(NEC):
                for dco in range(4):
                    nc.tensor.matmul(
                        ps2[:, b * NEC + ec: b * NEC + ec + 1],
                        lhsT=woutT[:, 4 + dco, ec * 128:(ec + 1) * 128],
                        rhs=outpcol[:, b * 4 + dco: b * 4 + dco + 1],
                        start=(dco == 0),
                        stop=(dco == 3),
                    )
        for b in range(BC):
            nc.vector.tensor_add(
                bias2sb[:, b * NEC:(b + 1) * NEC],
                ps2[:, b * NEC:(b + 1) * NEC],
                boutcol,
            )

    # ---- main loop over batches (tails deferred one batch for overlap) ----
    def emit_blocks(b, pre_energy_hook=None):
        ctx_tiles = []
        pcol = psum_pcol.tile([128, SCHUNKS], F32)
        lg_pending = [None]

        def logit_mms(kp, tanh_p):
            for half in range(2):
                for ss in range(NSS):
                    j = (kp * 2 + half) * NSS + ss
                    for ec in range(NEC):
                        nc.tensor.matmul(
                            pcol[:, j:j + 1],
                            lhsT=tanh_p[ec][:, half, ss * 128:(ss + 1) * 128],
                            rhs=w2colb[:, ec:ec + 1],
                            start=(ec == 0),
                            stop=(ec == NEC - 1),
                        )

        def load_and_transpose(k):
            # load one s-block [128, 4, 512], casting fp32 -> fp8e4 in the DMA
            if b == 0 and k in preloaded:
                ct = preloaded.pop(k)
            else:
                ct = start_load(b, k)

            ctxT = []
            for dp in range(NDC // 2):
                pt = psum_tp.tile([128, 2, SBLK, 2], FP8, tag="tp")
                for half in range(2):
                    dc = dp * 2 + half
                    for ss in range(NSS):
                        nc.tensor.transpose(
                            pt[:, half, ss * 128:(ss + 1) * 128, 0],
                            ct[:, ss, dc * 128:(dc + 1) * 128],
                            id128q,
                        )
                st = ctxT_pool.tile([128, 2, SBLK, 2], FP8, tag="ctxT")
                nc.vector.tensor_copy(
                    st.bitcast(mybir.dt.uint16), pt.bitcast(mybir.dt.uint16)
                )
                ctxT.append(st[:, :, :, 0])
            return ct, ctxT

        # transpose stage runs one block ahead so the energy matmuls never
        # wait on the DVE copyback of their own block; energy psum + tanh are
        # fused over PAIRS of s-blocks (2-bank psum tile, one ACT op per pair)
        cur = load_and_transpose(0)
        ctxT_pair = [None, None]
        for k in range(NSBLK):
            ct, ctxT = cur
            ctx_tiles.append(ct)
            ctxT_pair[k % 2] = ctxT
            if k % 2 == 0:
                # next (odd) block's transposes go ahead of the pair's energy
                # so the copyback of THIS block is hidden behind PE work
                cur = load_and_transpose(k + 1)
                if pre_energy_hook is not None and k in pre_energy_hook:
                    pre_energy_hook[k]()
                continue
            kp = k // 2

            # energyT[e_chunk, s_blk] = sum_pair W1c8[pair,ec].T @ ctxT[pair]
            # (fp8 DoubleRow: each matmul contracts 2 d-chunks = 256 rows)
            tanh_p = []
            for ec in range(NEC):
                pe = psum_en.tile([128, 2, SBLK], F32, tag="en")
                for half in range(2):
                    for pair in range(NDC // 2):
                        nc.tensor.matmul(
                            pe[:, half, :],
                            lhsT=w1c8sb[pair][:, :, ec * 128:(ec + 1) * 128],
                            rhs=ctxT_pair[half][pair],
                            start=(pair == 0),
                            stop=(pair == NDC // 2 - 1),
                            perf_mode=DOUBLE_ROW,
                        )
                th = tanh_pool.tile([128, 2, SBLK], BF16, tag="tanh")
                nc.scalar.activation(
                    th, pe, AF.Tanh,
                    bias=offsb[:, b * NEC + ec: b * NEC + ec + 1],
                    scale=1.0 / W1SCALE,
                )
                tanh_p.append(th)

            # logit matvecs for the PREVIOUS pair, so the PE's static order
            # never waits on a tanh that ACT has only just been issued
            if lg_pending[0] is not None:
                logit_mms(*lg_pending[0])
            lg_pending[0] = (kp, tanh_p)
            # the next (even) block's load+transposes ride after the energy
            if k + 1 < NSBLK:
                cur = load_and_transpose(k + 1)
            if pre_energy_hook is not None and k in pre_energy_hook:
                pre_energy_hook[k]()
        logit_mms(*lg_pending[0])

        # exp right away (frees pcol for the next batch; tail only needs pexp)
        pexp = small.tile([128, SCHUNKS], BF16, tag="pexp")
        rowsum = small.tile([128, 1], F32, tag="rowsum")
        nc.scalar.activation(pexp, pcol, AF.Exp, accum_out=rowsum)
        return ctx_tiles, pexp, rowsum

    def emit_tail(b, ctx_tiles, pexp, rowsum):
        # normalization chain (1/Z broadcast column) ...
        pd = psum_misc.tile([1, 1], F32, tag="misc")
        nc.tensor.matmul(pd, lhsT=rowsum, rhs=ones128)
        inv = small.tile([1, 1], F32, tag="inv")
        nc.vector.reciprocal(inv, pd)
        pinvb = psum_misc.tile([128, 1], F32, tag="misc")
        nc.tensor.matmul(pinvb, lhsT=ones_row, rhs=inv)
        invb = small.tile([128, 1], F32, tag="invb")
        nc.vector.tensor_copy(invb, pinvb)

        # ... runs in parallel with the unnormalized mix -> Wout matmuls:
        # mix columns directly: ctx chunks as stationary (LDW stream),
        # exp-weight column as the 1-wide moving operand; accumulate over j
        pmcol = psum_misc.tile([128, 4], F32, tag="misc")
        for dc in range(4):
            for j in range(SCHUNKS):
                nc.tensor.matmul(
                    pmcol[:, dc:dc + 1],
                    lhsT=ctx_tiles[j // NSS][:, j % NSS, dc * 128:(dc + 1) * 128],
                    rhs=pexp[:, j:j + 1],
                    start=(j == 0),
                    stop=(j == SCHUNKS - 1),
                )
        mc = small.tile([128, 4], BF16, tag="mc_sb")
        nc.vector.tensor_copy(mc, pmcol)

        # final: out_col[ec] = tanh(invZ * (WoutT[:, :4] @ mc) + bias2[b])
        pfo = psum_misc.tile([128, NEC], F32, tag="misc")
        for ec in range(NEC):
            for dc in range(4):
                nc.tensor.matmul(
                    pfo[:, ec:ec + 1],
                    lhsT=woutT[:, dc, ec * 128:(ec + 1) * 128],
                    rhs=mc[:, dc:dc + 1],
                    start=(dc == 0),
                    stop=(dc == 3),
                )
        fo = small.tile([128, NEC], F32, tag="fo_sb")
        for ec in range(NEC):
            nc.scalar.activation(
                fo[:, ec:ec + 1], pfo[:, ec:ec + 1], AF.Tanh,
                bias=bias2sb[:, b * NEC + ec: b * NEC + ec + 1],
                scale=invb,
            )

        # back to a row [1, 512] and out
        por = psum_misc.tile([1, HE], F32, tag="misc")
        for ec in range(NEC):
            nc.tensor.transpose(
                por[:, ec * 128:(ec + 1) * 128], fo[:, ec:ec + 1], id128f
            )
        orow = small.tile([1, HE], F32, tag="orow")
        nc.vector.tensor_copy(orow, por)
        nc.sync.dma_start(out=out_ap[b], in_=orow)

    def emit_late_setup():
        emit_tail_loads()
        emit_offs([1, 2, 3])
        emit_bias2()

    hooks0 = {1: lambda: emit_offs([0]), 5: emit_late_setup}
    pending = None
    for b in range(BC):
        state = emit_blocks(b, pre_energy_hook=hooks0 if b == 0 else None)
        if pending is not None:
            emit_tail(pending[0], *pending[1])
        pending = (b, state)
    emit_tail(pending[0], *pending[1])


INPUT_SPECS = {
    "context": ((BC, S, HE), F32),
    "blobA": ((128, 2208), mybir.dt.uint8),
    "blobB": ((128, 4104), mybir.dt.uint8),
    "blobC": ((128, 9240), mybir.dt.uint8),
}

_CACHE = {}


def build_nc():
    if "nc" in _CACHE:
        return _CACHE["nc"]
    nc = bacc.Bacc("TRN2", target_bir_lowering=False, debug=False,
                   num_devices=NCORES)
    ins = {
        name: nc.dram_tensor(name, list(shape), dt, kind="ExternalInput").ap()
        for name, (shape, dt) in INPUT_SPECS.items()
    }
    out = nc.dram_tensor("out", [BC, 1, HE], F32, kind="ExternalOutput").ap()
    with tile.TileContext(nc) as tc:
        attention_kernel(tc, out, ins)
    nc.compile()
    _CACHE["nc"] = nc
    return nc


def _u8(a):
    return np.ascontiguousarray(a).reshape(128, -1).view(np.uint8)


def make_in_maps(output, context, W1, W2, Wout, bout):
    """Shard the activations; precompute all weight-only layout transforms
    host-side (transposes, fp8 scaling + DoubleRow pairing, columnization)
    and pack them into three per-partition byte blobs (one DMA each)."""
    f8 = mybir.dt.np(FP8)
    bf = mybir.dt.np(BF16)
    W1 = np.asarray(W1, np.float32)
    Wout = np.asarray(Wout, np.float32)
    # W1 context half, transposed [d, e], scaled, DR-paired [pair, k, half, e]
    w1cT = W1[:, :HE].T
    w1c8 = np.clip(w1cT * W1SCALE, -240.0, 240.0).reshape(2, 2, 128, HE)
    w1c8 = np.ascontiguousarray(w1c8.transpose(0, 2, 1, 3)).astype(f8)
    # W1 output half, transposed [k, dco, e], bf16
    w1oT = np.ascontiguousarray(W1[:, HE:].T.reshape(NDC, 128, HE)
                                .transpose(1, 0, 2)).astype(bf)
    # Wout transposed [k, dc, e], bf16
    woutT = np.ascontiguousarray(Wout.T.reshape(8, 128, HE)
                                 .transpose(1, 0, 2)).astype(bf)
    w2col = np.ascontiguousarray(np.asarray(W2, np.float32)[0]
                                 .reshape(NEC, 128).T).astype(bf)
    boutcol = np.ascontiguousarray(np.asarray(bout, np.float32)
                                   .reshape(NEC, 128).T)
    id128f = np.eye(128, dtype=np.float32)
    id128q = id128f.astype(f8)
    onesv = np.ones((128, 130), np.float32)

    blobB = np.concatenate([_u8(w1oT), _u8(w2col)], axis=1)
    blobC = np.concatenate(
        [_u8(id128f), _u8(onesv), _u8(boutcol), _u8(woutT)], axis=1)

    # output rows columnized per core: outpcol[k, b*4+dc] = output[b,0,dc*128+k]
    outp = np.asarray(output, np.float32).reshape(B, 4, 128)
    maps = []
    for i in range(NCORES):
        sl = slice(i * BC, (i + 1) * BC)
        outpcol = np.ascontiguousarray(
            outp[sl].reshape(BC * 4, 128).T).astype(bf)
        blobA = np.concatenate(
            [_u8(id128q), _u8(w1c8[0]), _u8(w1c8[1]), _u8(outpcol)], axis=1)
        maps.append({
            "context": np.ascontiguousarray(context[sl], dtype=np.float32),
            "blobA": blobA,
            "blobB": blobB,
            "blobC": blobC,
        })
    return maps


def run(inputs, trace=False):
    from concourse.bass_utils import run_bass_kernel_spmd

    nc = build_nc()
    in_maps = make_in_maps(**inputs)
    res = run_bass_kernel_spmd(nc, in_maps, list(range(NCORES)), trace=trace)
    out = np.concatenate([res.results[i]["out"] for i in range(NCORES)], axis=0)
    return out, res


def kernel(output, context, W1, W2, Wout, bout):
    out, _ = run(dict(output=output, context=context, W1=W1, W2=W2,
                      Wout=Wout, bout=bout))
    return out
